# revision 5
# baseline (speedup 1.0000x reference)
import itertools
import numpy as np
import jax
import jax.numpy as jnp
from jax.sharding import Mesh, PartitionSpec
from jax.experimental.shard_map import shard_map
from functools import partial

# Problem constants (hardcoded per contract)
D = 3
N_LEVELS = 16
F = 2
LOG2_T = 19
TABLE_SIZE = 1 << LOG2_T
BASE_RES = 16.0
FINEST_RES = 512.0
N_POINTS = 1_000_000
N_CORES = 8
PRIMES = np.array([1, 2654435761, 805459861], dtype=np.uint32)
OFFSETS = np.array(list(itertools.product([0, 1], repeat=D)), dtype=np.float32)

_RES = []
_b = np.exp((np.log(FINEST_RES) - np.log(BASE_RES)) / (N_LEVELS - 1))
for i in range(N_LEVELS):
    _RES.append(float(np.floor(np.float32(BASE_RES) * np.float32(_b) ** i)))

_BOX_MIN = np.full((D,), -1.0, np.float32)
_BOX_MAX = np.full((D,), 1.0, np.float32)


def _hash_encode_level(x, table, resolution):
    box_min = jnp.asarray(_BOX_MIN)
    box_max = jnp.asarray(_BOX_MAX)
    xc = jnp.clip(x, box_min, box_max)
    grid = (box_max - box_min) / jnp.float32(resolution)
    bl = jnp.floor((xc - box_min) / grid)
    vmin = bl * grid + box_min
    vmax = vmin + grid
    verts = bl.astype(jnp.uint32)[:, None, :] + jnp.asarray(OFFSETS, jnp.uint32)[None]
    h = verts * jnp.asarray(PRIMES)[None, None, :]
    idx = (h[..., 0] ^ h[..., 1] ^ h[..., 2]) & jnp.uint32(TABLE_SIZE - 1)
    emb = table[idx]
    w = (xc - vmin) / (vmax - vmin)
    mask = jnp.asarray(OFFSETS, bool)[None]
    wc = jnp.prod(jnp.where(mask, w[:, None, :], jnp.float32(1.0)), axis=-1)
    return jnp.einsum('nv,nvf->nf', wc, emb)


def _forward_shard(x, tables):
    # x: [N/8, D] local shard; tables: [N_LEVELS, T, F] replicated
    feats = []
    for i in range(N_LEVELS):
        feats.append(_hash_encode_level(x, tables[i], _RES[i]))
    return jnp.concatenate(feats, axis=-1)


_cached = {}

# points per core per NEFF call; keeps per-NEFF gather-instruction count
# (CHUNK*16*8 per core) under the neuronx-cc 5M instruction ceiling.
CHUNK = 4096


def _get_jitted():
    if "fn" in _cached:
        return _cached["fn"], _cached["mesh"]
    devices = jax.devices()[:N_CORES]
    mesh = Mesh(np.asarray(devices), ("core",))
    fn = jax.jit(
        shard_map(
            _forward_shard,
            mesh=mesh,
            in_specs=(PartitionSpec("core"), PartitionSpec()),
            out_specs=PartitionSpec("core"),
            check_rep=False,
        )
    )
    _cached["fn"] = fn
    _cached["mesh"] = mesh
    return fn, mesh


def kernel(x, tables):
    x = np.asarray(x, dtype=np.float32)
    tables = np.asarray(tables, dtype=np.float32)
    n = x.shape[0]
    per_core = (n + N_CORES - 1) // N_CORES          # 125000
    n_chunks = (per_core + CHUNK - 1) // CHUNK
    pad_per_core = n_chunks * CHUNK                  # padded points per core
    # lay out as [N_CORES, pad_per_core, D] so each device's shard stays its own
    xs = np.zeros((N_CORES, pad_per_core, D), np.float32)
    for c in range(N_CORES):
        lo, hi = c * per_core, min((c + 1) * per_core, n)
        xs[c, : hi - lo] = x[lo:hi]
    fn, mesh = _get_jitted()
    from jax.sharding import NamedSharding
    tab = jax.device_put(tables, NamedSharding(mesh, PartitionSpec()))
    outs = np.empty((N_CORES, pad_per_core, N_LEVELS * F), np.float32)
    # queue all chunk executions asynchronously, then materialize — lets jax
    # overlap host transfers with device execution across chunks
    pending = []
    for k in range(n_chunks):
        xc = xs[:, k * CHUNK:(k + 1) * CHUNK].reshape(N_CORES * CHUNK, D)
        pending.append(fn(xc, tab))                  # [N_CORES*CHUNK, 32]
    for k, o in enumerate(pending):
        o = np.asarray(o)
        outs[:, k * CHUNK:(k + 1) * CHUNK] = o.reshape(N_CORES, CHUNK, -1)
    out = np.empty((n, N_LEVELS * F), np.float32)
    for c in range(N_CORES):
        lo, hi = c * per_core, min((c + 1) * per_core, n)
        out[lo:hi] = outs[c, : hi - lo]
    return out
